# revision 15
# baseline (speedup 1.0000x reference)
"""FBSNN net_u_Du kernel for 8 trn2 NeuronCores.

Computes, for u(s) = W2 @ sin(W1 @ s + b1) + b2 with s = [t, x]:
  u            (M,1)
  DuDx = g[:,1:], DuDt = g[:,:1]  with  g = (W2 o cos Z) @ W1
  D2uDx2[m]    = V^T diag(-W2 o sin z_m) V,  V = W1[:,1:]

Key reductions:
 - the per-sample Hessians batch into one dense matmul
     D2[m, jk] = sum_h sin(Z)[h,m] * P[h, jk],  P = -W2 (x) V (x) V
 - the Hessian is symmetric, so only the block-upper-triangle is computed
   on device (55 of 100 10x10 blocks, packed into 5500 columns); the host
   mirrors the lower blocks.
Data parallel over M=4096 paths -> 512 per core; weights replicated.

HW notes this shape leans on:
 - HW Sin is only accurate on [-pi, pi]; arguments are range-reduced with
   w = y - 2pi*round(y/2pi) (the DVE f32->i32 cast rounds to nearest).
 - fp32 matmul runs as two PE passes with serializing hi/lo weight loads;
   the Hessian matmul uses fp16 operands (full-rate streaming, fp32 PSUM
   accumulation). P carries a 2^10 scale so its entries stay in the fp16
   normal range; the PSUM->SBUF copy divides it back out (exact).
 - all inputs are packed into one (128, 976) tensor loaded by a single
   SWDGE DMA; HWDGE 2D loads serialize on one SDMA engine (~15us).
"""

import numpy as np

import concourse.bacc as bacc
import concourse.mybir as mybir
import concourse.tile as tile
from concourse.bass_utils import run_bass_kernel_spmd

N_CORES = 8
M_FULL = 4096
MC = M_FULL // N_CORES  # 512 paths per core
D = 100
DP1 = D + 1  # 101
H = 256  # hidden width
F32 = mybir.dt.float32
F16 = mybir.dt.float16

BS = 10  # hessian block size
NJB = D // BS  # 10 j-blocks
# packed block-upper-triangle: j-block jb covers columns k in [10*jb, 100)
WIDTHS = [D - BS * jb for jb in range(NJB)]  # 100, 90, ..., 10
OFFS = np.cumsum([0] + [BS * w for w in WIDTHS]).tolist()  # packed offsets
PCOLS = OFFS[-1]  # 5500
PSCALE = 1024.0  # 2^10: keeps fp16 P entries in normal range

NCHUNK = 500  # matmul free-dim (<=512 fp32 PSUM bank)
NCH = PCOLS // NCHUNK  # 11 chunks
DMA_GRPS = [(0, 4), (4, 8), (8, 11)]  # chunk ranges per output DMA
NM = MC // 128  # 4 m-chunks of 128 paths

# set by test harness to profile; kernel() records exec time here
TRACE = False
LAST_EXEC_NS = None

_CACHE = {}


def _build():
    nc = bacc.Bacc(None, target_bir_lowering=False, debug=False)
    sin_f = mybir.ActivationFunctionType.Sin
    copy_f = mybir.ActivationFunctionType.Copy
    mult = mybir.AluOpType.mult

    # One packed input, (128, 976):
    #   cols 0:768    [XT | W1T] content on partitions 0:101 (rest zero)
    #   cols 768:872  [W1[0:128] | b1 | W2 | b2pad]
    #   cols 872:976  [W1[128:256] | b1 | W2 | 0]
    CBW = DP1 + 3
    in_d = nc.dram_tensor("IN", [128, MC + H + 2 * CBW], F32, kind="ExternalInput")

    u_d = nc.dram_tensor("u", [1, MC], F32, kind="ExternalOutput")
    gp_d = nc.dram_tensor("GP", [128, NM * DP1], F32, kind="ExternalOutput")
    d2_d = nc.dram_tensor("D2P", [MC, PCOLS], F32, kind="ExternalOutput")

    with tile.TileContext(nc) as tc:
        with (
            tc.tile_pool(name="const", bufs=1) as const,
            tc.tile_pool(name="work", bufs=2) as work,
            tc.tile_pool(name="stage", bufs=4) as stage_p,
            tc.tile_pool(name="psA", bufs=2, space="PSUM") as psA,
        ):
            # ---- load inputs: one sprayed DMA ----
            inp = const.tile([128, MC + H + 2 * CBW], F32)
            nc.gpsimd.dma_start(inp[:], in_d[:])
            ca = inp[0:DP1, :]
            cb = [inp[:, MC + H + k * CBW : MC + H + (k + 1) * CBW] for k in range(2)]
            xt = ca[:, 0:MC]
            w1t = ca[:, MC : MC + H]
            w1 = [cb[k][:, 0:DP1] for k in range(2)]
            b1c = [cb[k][:, DP1 : DP1 + 1] for k in range(2)]
            w2c = [cb[k][:, DP1 + 1 : DP1 + 2] for k in range(2)]
            b2t = cb[0][0:1, DP1 + 2 : DP1 + 3]

            pihalf = const.tile([128, 1], F32)
            nc.vector.memset(pihalf[:], float(np.pi / 2))

            # w1w2 = W2 o W1 rows (g matmul rhs); fp16 V and -1024*W2*V for P
            w1w2 = [const.tile([128, DP1], F32, tag=f"w1w2_{k}", name=f"w1w2_{k}") for k in range(2)]
            vv16 = [const.tile([128, D], F16, tag=f"vv16_{k}", name=f"vv16_{k}") for k in range(2)]
            nw2v16 = [const.tile([128, D], F16, tag=f"nw2v16_{k}", name=f"nw2v16_{k}") for k in range(2)]
            for k in range(2):
                nc.vector.tensor_scalar_mul(w1w2[k][:], w1[k][:], w2c[k])
                nc.vector.tensor_copy(vv16[k][:], w1[k][:, 1:DP1])
                nc.vector.tensor_scalar_mul(
                    nw2v16[k][:], w1w2[k][:, 1:DP1], -PSCALE
                )

            # trig outputs
            sh16 = [const.tile([128, MC], F16, tag=f"sh16_{k}", name=f"sh16_{k}") for k in range(2)]
            sin32 = [const.tile([128, MC], F32, tag=f"sin32_{k}", name=f"sin32_{k}") for k in range(2)]
            cos32 = [const.tile([128, MC], F32, tag=f"cos32_{k}", name=f"cos32_{k}") for k in range(2)]
            inv2pi = float(1.0 / (2.0 * np.pi))
            twopi = float(2.0 * np.pi)

            # ---- trig: Z^T = W1 @ [t,X]^T; sin/cos via range-reduced Sin ----
            for k in range(2):
                ztp = psA.tile([128, MC], F32, tag="zt")
                nc.tensor.matmul(
                    ztp[:], w1t[:, k * 128 : (k + 1) * 128], xt[:],
                    start=True, stop=True,
                )
                y = work.tile([128, MC], F32, tag="y")
                nc.vector.tensor_scalar_add(y[:], ztp[:], b1c[k])
                ki = work.tile([128, MC], mybir.dt.int32, tag="ki")
                nc.vector.tensor_scalar(
                    out=ki[:], in0=y[:], scalar1=inv2pi, scalar2=None, op0=mult
                )
                kf = work.tile([128, MC], F32, tag="kf")
                nc.vector.tensor_scalar(
                    out=kf[:], in0=ki[:], scalar1=twopi, scalar2=None, op0=mult
                )
                w = work.tile([128, MC], F32, tag="wred")
                nc.vector.tensor_tensor(
                    out=w[:], in0=y[:], in1=kf[:], op=mybir.AluOpType.subtract
                )
                nc.scalar.activation(sh16[k][:], w[:], sin_f)
                nc.scalar.activation(sin32[k][:], w[:], sin_f)
                # cos(z) = sin(w + pi/2), wrapped down a period if w > pi/2
                hi = work.tile([128, MC], F32, tag="hi")
                nc.vector.tensor_scalar(
                    out=hi[:], in0=w[:], scalar1=float(np.pi / 2), scalar2=-twopi,
                    op0=mybir.AluOpType.is_gt, op1=mult,
                )
                wc = work.tile([128, MC], F32, tag="wc")
                nc.vector.tensor_tensor(
                    out=wc[:], in0=w[:], in1=hi[:], op=mybir.AluOpType.add
                )
                nc.scalar.activation(cos32[k][:], wc[:], sin_f, bias=pihalf[:])

            # packed block-triangle P (built once, fp16, persistent)
            PP = [const.tile([128, PCOLS], F16, tag=f"PP_{k}", name=f"PP_{k}") for k in range(2)]
            for k in range(2):
                for jb in range(NJB):
                    wjb = WIDTHS[jb]
                    js = slice(jb * BS, (jb + 1) * BS)
                    nc.vector.tensor_tensor(
                        out=PP[k][:, OFFS[jb] : OFFS[jb] + BS * wjb].rearrange(
                            "p (j l) -> p j l", l=wjb
                        ),
                        in0=nw2v16[k][:, js, None].to_broadcast([128, BS, wjb]),
                        in1=vv16[k][:, None, jb * BS : D].to_broadcast([128, BS, wjb]),
                        op=mult,
                    )

            # ---- Hessian: D2P[m, c] = (1/1024) sum_h sin16[h,m] * PP[h, c],
            # interleaved with the small u and g matmuls so the big output
            # DMA starts as early as possible and u/g fill PE copy-gaps.
            g_all = const.tile([128, NM * DP1], F32)

            def emit_hess(m):
                ms = slice(m * 128, (m + 1) * 128)
                for g0, g1 in DMA_GRPS:
                    gcols = (g1 - g0) * NCHUNK
                    st = stage_p.tile([128, 2000], F32, tag="stage", name=f"st_{m}_{g0}")
                    pss = [
                        psA.tile([128, NCHUNK], F32, tag="hess", bufs=5,
                                 name=f"ps_{m}_{c}")
                        for c in range(g0, g1)
                    ]
                    for k in range(2):
                        for i, c in enumerate(range(g0, g1)):
                            cs = slice(c * NCHUNK, (c + 1) * NCHUNK)
                            nc.tensor.matmul(
                                pss[i][:], sh16[k][:, ms], PP[k][:, cs],
                                start=(k == 0), stop=(k == 1),
                            )
                    for i, c in enumerate(range(g0, g1)):
                        ss = slice(i * NCHUNK, (i + 1) * NCHUNK)
                        if i % 2 == 0:
                            nc.vector.tensor_scalar_mul(
                                st[:, ss], pss[i][:], 1.0 / PSCALE
                            )
                        else:
                            nc.scalar.activation(
                                st[:, ss], pss[i][:], copy_f, scale=1.0 / PSCALE
                            )
                    nc.sync.dma_start(
                        d2_d[ms, g0 * NCHUNK : g1 * NCHUNK], st[:, 0:gcols]
                    )

            def emit_u():
                up = psA.tile([1, MC], F32, tag="ug", bufs=1, name="up")
                for k in range(2):
                    nc.tensor.matmul(
                        up[:], w2c[k], sin32[k][:], start=(k == 0), stop=(k == 1)
                    )
                u_sb = work.tile([1, MC], F32, tag="usb", name="u_sb")
                nc.vector.tensor_scalar_add(u_sb[:], up[:], b2t)
                nc.sync.dma_start(u_d[:], u_sb[:])

            def emit_g(m):
                ms = slice(m * 128, (m + 1) * 128)
                gp = psA.tile([128, DP1], F32, tag="ug", bufs=1, name=f"gp_{m}")
                for k in range(2):
                    nc.tensor.matmul(
                        gp[:], cos32[k][:, ms], w1w2[k][:], start=(k == 0), stop=(k == 1)
                    )
                nc.vector.tensor_copy(g_all[:, m * DP1 : (m + 1) * DP1], gp[:])

            emit_hess(0)
            emit_u()
            emit_hess(1)
            emit_g(0)
            emit_g(1)
            emit_hess(2)
            emit_g(2)
            emit_g(3)
            nc.sync.dma_start(gp_d[:], g_all[:])
            emit_hess(3)

    nc.compile()
    return nc


def kernel(t, X, W1, b1, W2, b2):
    global LAST_EXEC_NS
    t = np.ascontiguousarray(np.asarray(t, dtype=np.float32))
    X = np.ascontiguousarray(np.asarray(X, dtype=np.float32))
    W1 = np.ascontiguousarray(np.asarray(W1, dtype=np.float32))
    b1 = np.asarray(b1, dtype=np.float32).reshape(H)
    W2 = np.asarray(W2, dtype=np.float32).reshape(H)
    b2 = np.asarray(b2, dtype=np.float32).reshape(1)

    xaug_t = np.concatenate([t, X], axis=1).T  # (101, 4096)
    w1t = W1.T  # (101, 256)

    CBW = DP1 + 3
    base = np.zeros((128, MC + H + 2 * CBW), dtype=np.float32)
    base[0:DP1, MC : MC + H] = w1t
    for k in range(2):
        c0 = MC + H + k * CBW
        base[:, c0 : c0 + DP1] = W1[k * 128 : (k + 1) * 128, :]
        base[:, c0 + DP1] = b1[k * 128 : (k + 1) * 128]
        base[:, c0 + DP1 + 1] = W2[k * 128 : (k + 1) * 128]
    base[0, MC + H + DP1 + 2] = b2[0]

    if "nc" not in _CACHE:
        _CACHE["nc"] = _build()
    nc = _CACHE["nc"]

    in_maps = []
    for i in range(N_CORES):
        pk = base.copy()
        pk[0:DP1, 0:MC] = xaug_t[:, i * MC : (i + 1) * MC]
        in_maps.append({"IN": pk})

    res = run_bass_kernel_spmd(nc, in_maps, list(range(N_CORES)), trace=TRACE)
    LAST_EXEC_NS = res.exec_time_ns

    u = np.concatenate(
        [res.results[i]["u"].reshape(MC, 1) for i in range(N_CORES)], axis=0
    )
    g = np.concatenate(
        [
            res.results[i]["GP"].reshape(128, NM, DP1).transpose(1, 0, 2).reshape(MC, DP1)
            for i in range(N_CORES)
        ],
        axis=0,
    )
    dudt = np.ascontiguousarray(g[:, 0:1])
    dudx = np.ascontiguousarray(g[:, 1:DP1])
    packed = np.concatenate([res.results[i]["D2P"] for i in range(N_CORES)], axis=0)

    # unpack block-upper-triangle and mirror (Hessian is symmetric)
    d2 = np.empty((M_FULL, D, D), dtype=np.float32)
    for jb in range(NJB):
        wjb = WIDTHS[jb]
        blk = packed[:, OFFS[jb] : OFFS[jb] + BS * wjb].reshape(M_FULL, BS, wjb)
        d2[:, jb * BS : (jb + 1) * BS, jb * BS : D] = blk
    for jb in range(1, NJB):
        for kb in range(jb):
            d2[:, jb * BS : (jb + 1) * BS, kb * BS : (kb + 1) * BS] = d2[
                :, kb * BS : (kb + 1) * BS, jb * BS : (jb + 1) * BS
            ].transpose(0, 2, 1)
    return u, dudx, dudt, d2


# revision 16
# speedup vs baseline: 1.1184x; 1.1184x over previous
"""FBSNN net_u_Du kernel for 8 trn2 NeuronCores.

Computes, for u(s) = W2 @ sin(W1 @ s + b1) + b2 with s = [t, x]:
  u            (M,1)
  DuDx = g[:,1:], DuDt = g[:,:1]  with  g = (W2 o cos Z) @ W1
  D2uDx2[m]    = V^T diag(-W2 o sin z_m) V,  V = W1[:,1:]

Key reductions:
 - the per-sample Hessians batch into one dense matmul
     D2[m, jk] = sum_h sin(Z)[h,m] * P[h, jk],  P = -W2 (x) V (x) V
 - the Hessian is symmetric, so only the block-upper-triangle is computed
   on device (55 of 100 10x10 blocks, packed into 5500 columns); the host
   mirrors the lower blocks.
Data parallel over M=4096 paths -> 512 per core; weights replicated.

HW notes this shape leans on:
 - HW Sin is only accurate on [-pi, pi]; arguments are range-reduced with
   w = y - 2pi*round(y/2pi) (the DVE f32->i32 cast rounds to nearest).
 - fp32 matmul runs as two PE passes with serializing hi/lo weight loads;
   the Hessian matmul uses fp16 operands (full-rate streaming, fp32 PSUM
   accumulation). P carries a 2^10 scale so its entries stay in the fp16
   normal range; the PSUM->SBUF copy divides it back out (exact).
 - all inputs are packed into one (128, 976) tensor loaded by a single
   SWDGE DMA; HWDGE 2D loads serialize on one SDMA engine (~15us).
"""

import numpy as np

import concourse.bacc as bacc
import concourse.mybir as mybir
import concourse.tile as tile
from concourse.bass_utils import run_bass_kernel_spmd

N_CORES = 8
M_FULL = 4096
MC = M_FULL // N_CORES  # 512 paths per core
D = 100
DP1 = D + 1  # 101
H = 256  # hidden width
F32 = mybir.dt.float32
F16 = mybir.dt.float16

BS = 10  # hessian block size
NJB = D // BS  # 10 j-blocks
# packed block-upper-triangle: j-block jb covers columns k in [10*jb, 100)
WIDTHS = [D - BS * jb for jb in range(NJB)]  # 100, 90, ..., 10
OFFS = np.cumsum([0] + [BS * w for w in WIDTHS]).tolist()  # packed offsets
PCOLS = OFFS[-1]  # 5500
PSCALE = 1024.0  # 2^10: keeps fp16 P entries in normal range

NCHUNK = 500  # matmul free-dim (<=512 fp32 PSUM bank)
NCH = PCOLS // NCHUNK  # 11 chunks
DMA_GRPS = [(0, 4), (4, 8), (8, 11)]  # chunk ranges per output DMA
NM = MC // 128  # 4 m-chunks of 128 paths

# set by test harness to profile; kernel() records exec time here
TRACE = False
LAST_EXEC_NS = None

_CACHE = {}


def _build():
    nc = bacc.Bacc(None, target_bir_lowering=False, debug=False)
    sin_f = mybir.ActivationFunctionType.Sin
    copy_f = mybir.ActivationFunctionType.Copy
    mult = mybir.AluOpType.mult

    # One packed input, (128, 976):
    #   cols 0:768    [XT | W1T] content on partitions 0:101 (rest zero)
    #   cols 768:872  [W1[0:128] | b1 | W2 | b2pad]
    #   cols 872:976  [W1[128:256] | b1 | W2 | 0]
    CBW = DP1 + 3
    in_d = nc.dram_tensor("IN", [128, MC + H + 2 * CBW], F32, kind="ExternalInput")

    u_d = nc.dram_tensor("u", [1, MC], F32, kind="ExternalOutput")
    gp_d = nc.dram_tensor("GP", [128, NM * DP1], F32, kind="ExternalOutput")
    d2_d = nc.dram_tensor("D2P", [MC, PCOLS], F32, kind="ExternalOutput")

    with tile.TileContext(nc) as tc:
        with (
            tc.tile_pool(name="const", bufs=1) as const,
            tc.tile_pool(name="work", bufs=2) as work,
            tc.tile_pool(name="stage", bufs=4) as stage_p,
            tc.tile_pool(name="psA", bufs=2, space="PSUM") as psA,
        ):
            # ---- load inputs: one sprayed DMA ----
            inp = const.tile([128, MC + H + 2 * CBW], F32)
            nc.gpsimd.dma_start(inp[:], in_d[:])
            ca = inp[0:DP1, :]
            cb = [inp[:, MC + H + k * CBW : MC + H + (k + 1) * CBW] for k in range(2)]
            xt = ca[:, 0:MC]
            w1t = ca[:, MC : MC + H]
            w1 = [cb[k][:, 0:DP1] for k in range(2)]
            b1c = [cb[k][:, DP1 : DP1 + 1] for k in range(2)]
            w2c = [cb[k][:, DP1 + 1 : DP1 + 2] for k in range(2)]
            b2t = cb[0][0:1, DP1 + 2 : DP1 + 3]

            pihalf = const.tile([128, 1], F32)
            nc.vector.memset(pihalf[:], float(np.pi / 2))

            # w1w2 = W2 o W1 rows (g matmul rhs); fp16 V and -1024*W2*V for P
            w1w2 = [const.tile([128, DP1], F32, tag=f"w1w2_{k}", name=f"w1w2_{k}") for k in range(2)]
            vv16 = [const.tile([128, D], F16, tag=f"vv16_{k}", name=f"vv16_{k}") for k in range(2)]
            nw2v16 = [const.tile([128, D], F16, tag=f"nw2v16_{k}", name=f"nw2v16_{k}") for k in range(2)]
            for k in range(2):
                nc.vector.tensor_scalar_mul(w1w2[k][:], w1[k][:], w2c[k])
                nc.vector.tensor_copy(vv16[k][:], w1[k][:, 1:DP1])
                nc.vector.tensor_scalar_mul(
                    nw2v16[k][:], w1w2[k][:, 1:DP1], -PSCALE
                )

            # trig outputs
            sh16 = [const.tile([128, MC], F16, tag=f"sh16_{k}", name=f"sh16_{k}") for k in range(2)]
            sin32 = [const.tile([128, MC], F32, tag=f"sin32_{k}", name=f"sin32_{k}") for k in range(2)]
            cos32 = [const.tile([128, MC], F32, tag=f"cos32_{k}", name=f"cos32_{k}") for k in range(2)]
            inv2pi = float(1.0 / (2.0 * np.pi))
            twopi = float(2.0 * np.pi)

            # ---- trig: Z^T = W1 @ [t,X]^T; sin/cos via range-reduced Sin ----
            for k in range(2):
                ztp = psA.tile([128, MC], F32, tag="zt")
                nc.tensor.matmul(
                    ztp[:], w1t[:, k * 128 : (k + 1) * 128], xt[:],
                    start=True, stop=True,
                )
                y = work.tile([128, MC], F32, tag="y")
                nc.vector.tensor_scalar_add(y[:], ztp[:], b1c[k])
                ki = work.tile([128, MC], mybir.dt.int32, tag="ki")
                nc.vector.tensor_scalar(
                    out=ki[:], in0=y[:], scalar1=inv2pi, scalar2=None, op0=mult
                )
                kf = work.tile([128, MC], F32, tag="kf")
                nc.vector.tensor_scalar(
                    out=kf[:], in0=ki[:], scalar1=twopi, scalar2=None, op0=mult
                )
                w = work.tile([128, MC], F32, tag="wred")
                nc.vector.tensor_tensor(
                    out=w[:], in0=y[:], in1=kf[:], op=mybir.AluOpType.subtract
                )
                nc.scalar.activation(sh16[k][:], w[:], sin_f)
                nc.scalar.activation(sin32[k][:], w[:], sin_f)
                # cos(z) = sin(w + pi/2), wrapped down a period if w > pi/2
                hi = work.tile([128, MC], F32, tag="hi")
                nc.vector.tensor_scalar(
                    out=hi[:], in0=w[:], scalar1=float(np.pi / 2), scalar2=-twopi,
                    op0=mybir.AluOpType.is_gt, op1=mult,
                )
                wc = work.tile([128, MC], F32, tag="wc")
                nc.vector.tensor_tensor(
                    out=wc[:], in0=w[:], in1=hi[:], op=mybir.AluOpType.add
                )
                nc.scalar.activation(cos32[k][:], wc[:], sin_f, bias=pihalf[:])

            # packed block-triangle P (fp16, persistent); built just-in-time
            # inside the first m-chunk's groups so copies aren't starved.
            PP = [const.tile([128, PCOLS], F16, tag=f"PP_{k}", name=f"PP_{k}") for k in range(2)]

            def build_pp(jbs):
                for k in range(2):
                    for jb in jbs:
                        wjb = WIDTHS[jb]
                        js = slice(jb * BS, (jb + 1) * BS)
                        nc.vector.tensor_tensor(
                            out=PP[k][:, OFFS[jb] : OFFS[jb] + BS * wjb].rearrange(
                                "p (j l) -> p j l", l=wjb
                            ),
                            in0=nw2v16[k][:, js, None].to_broadcast([128, BS, wjb]),
                            in1=vv16[k][:, None, jb * BS : D].to_broadcast([128, BS, wjb]),
                            op=mult,
                        )

            GRP_JBS = {0: [0, 1, 2], 4: [3, 4], 8: [5, 6, 7, 8, 9]}

            # ---- Hessian: D2P[m, c] = (1/1024) sum_h sin16[h,m] * PP[h, c],
            # interleaved with the small u and g matmuls so the big output
            # DMA starts as early as possible and u/g fill PE copy-gaps.
            g_all = const.tile([128, NM * DP1], F32)

            def emit_hess(m):
                ms = slice(m * 128, (m + 1) * 128)
                for g0, g1 in DMA_GRPS:
                    if m == 0:
                        build_pp(GRP_JBS[g0])
                    gcols = (g1 - g0) * NCHUNK
                    st = stage_p.tile([128, 2000], F32, tag="stage", name=f"st_{m}_{g0}")
                    pss = [
                        psA.tile([128, NCHUNK], F32, tag="hess", bufs=5,
                                 name=f"ps_{m}_{c}")
                        for c in range(g0, g1)
                    ]
                    for k in range(2):
                        for i, c in enumerate(range(g0, g1)):
                            cs = slice(c * NCHUNK, (c + 1) * NCHUNK)
                            nc.tensor.matmul(
                                pss[i][:], sh16[k][:, ms], PP[k][:, cs],
                                start=(k == 0), stop=(k == 1),
                            )
                    for i, c in enumerate(range(g0, g1)):
                        ss = slice(i * NCHUNK, (i + 1) * NCHUNK)
                        if i == 0:
                            nc.vector.tensor_scalar_mul(
                                st[:, ss], pss[i][:], 1.0 / PSCALE
                            )
                        else:
                            nc.scalar.activation(
                                st[:, ss], pss[i][:], copy_f, scale=1.0 / PSCALE
                            )
                    nc.sync.dma_start(
                        d2_d[ms, g0 * NCHUNK : g1 * NCHUNK], st[:, 0:gcols]
                    )

            def emit_u():
                up = psA.tile([1, MC], F32, tag="ug", bufs=1, name="up")
                for k in range(2):
                    nc.tensor.matmul(
                        up[:], w2c[k], sin32[k][:], start=(k == 0), stop=(k == 1)
                    )
                u_sb = work.tile([1, MC], F32, tag="usb", name="u_sb")
                nc.vector.tensor_scalar_add(u_sb[:], up[:], b2t)
                nc.sync.dma_start(u_d[:], u_sb[:])

            def emit_g(m):
                ms = slice(m * 128, (m + 1) * 128)
                gp = psA.tile([128, DP1], F32, tag="ug", bufs=1, name=f"gp_{m}")
                for k in range(2):
                    nc.tensor.matmul(
                        gp[:], cos32[k][:, ms], w1w2[k][:], start=(k == 0), stop=(k == 1)
                    )
                nc.vector.tensor_copy(g_all[:, m * DP1 : (m + 1) * DP1], gp[:])

            emit_hess(0)
            emit_u()
            emit_hess(1)
            emit_g(0)
            emit_g(1)
            emit_hess(2)
            emit_g(2)
            emit_g(3)
            nc.sync.dma_start(gp_d[:], g_all[:])
            emit_hess(3)

    nc.compile()
    return nc


def kernel(t, X, W1, b1, W2, b2):
    global LAST_EXEC_NS
    t = np.ascontiguousarray(np.asarray(t, dtype=np.float32))
    X = np.ascontiguousarray(np.asarray(X, dtype=np.float32))
    W1 = np.ascontiguousarray(np.asarray(W1, dtype=np.float32))
    b1 = np.asarray(b1, dtype=np.float32).reshape(H)
    W2 = np.asarray(W2, dtype=np.float32).reshape(H)
    b2 = np.asarray(b2, dtype=np.float32).reshape(1)

    xaug_t = np.concatenate([t, X], axis=1).T  # (101, 4096)
    w1t = W1.T  # (101, 256)

    CBW = DP1 + 3
    base = np.zeros((128, MC + H + 2 * CBW), dtype=np.float32)
    base[0:DP1, MC : MC + H] = w1t
    for k in range(2):
        c0 = MC + H + k * CBW
        base[:, c0 : c0 + DP1] = W1[k * 128 : (k + 1) * 128, :]
        base[:, c0 + DP1] = b1[k * 128 : (k + 1) * 128]
        base[:, c0 + DP1 + 1] = W2[k * 128 : (k + 1) * 128]
    base[0, MC + H + DP1 + 2] = b2[0]

    if "nc" not in _CACHE:
        _CACHE["nc"] = _build()
    nc = _CACHE["nc"]

    in_maps = []
    for i in range(N_CORES):
        pk = base.copy()
        pk[0:DP1, 0:MC] = xaug_t[:, i * MC : (i + 1) * MC]
        in_maps.append({"IN": pk})

    res = run_bass_kernel_spmd(nc, in_maps, list(range(N_CORES)), trace=TRACE)
    LAST_EXEC_NS = res.exec_time_ns

    u = np.concatenate(
        [res.results[i]["u"].reshape(MC, 1) for i in range(N_CORES)], axis=0
    )
    g = np.concatenate(
        [
            res.results[i]["GP"].reshape(128, NM, DP1).transpose(1, 0, 2).reshape(MC, DP1)
            for i in range(N_CORES)
        ],
        axis=0,
    )
    dudt = np.ascontiguousarray(g[:, 0:1])
    dudx = np.ascontiguousarray(g[:, 1:DP1])
    packed = np.concatenate([res.results[i]["D2P"] for i in range(N_CORES)], axis=0)

    # unpack block-upper-triangle and mirror (Hessian is symmetric)
    d2 = np.empty((M_FULL, D, D), dtype=np.float32)
    for jb in range(NJB):
        wjb = WIDTHS[jb]
        blk = packed[:, OFFS[jb] : OFFS[jb] + BS * wjb].reshape(M_FULL, BS, wjb)
        d2[:, jb * BS : (jb + 1) * BS, jb * BS : D] = blk
    for jb in range(1, NJB):
        for kb in range(jb):
            d2[:, jb * BS : (jb + 1) * BS, kb * BS : (kb + 1) * BS] = d2[
                :, kb * BS : (kb + 1) * BS, jb * BS : (jb + 1) * BS
            ].transpose(0, 2, 1)
    return u, dudx, dudt, d2


# revision 17
# speedup vs baseline: 1.2520x; 1.1194x over previous
"""FBSNN net_u_Du kernel for 8 trn2 NeuronCores.

Computes, for u(s) = W2 @ sin(W1 @ s + b1) + b2 with s = [t, x]:
  u            (M,1)
  DuDx = g[:,1:], DuDt = g[:,:1]  with  g = (W2 o cos Z) @ W1
  D2uDx2[m]    = V^T diag(-W2 o sin z_m) V,  V = W1[:,1:]

Key reductions:
 - the per-sample Hessians batch into one dense matmul
     D2[m, jk] = sum_h sin(Z)[h,m] * P[h, jk],  P = -W2 (x) V (x) V
 - the Hessian is symmetric, so only the block-upper-triangle is computed
   on device (55 of 100 10x10 blocks, packed into 5500 columns); the host
   mirrors the lower blocks.
Data parallel over M=4096 paths -> 512 per core; weights replicated.

HW notes this shape leans on:
 - HW Sin is only accurate on [-pi, pi]; arguments are range-reduced with
   w = y - 2pi*round(y/2pi) (the DVE f32->i32 cast rounds to nearest).
 - fp32 matmul runs as two PE passes with serializing hi/lo weight loads;
   the Hessian matmul uses fp16 operands (full-rate streaming, fp32 PSUM
   accumulation). P carries a 2^10 scale so its entries stay in the fp16
   normal range; the PSUM->SBUF copy divides it back out (exact).
 - all inputs are packed into one (128, 976) tensor loaded by a single
   SWDGE DMA; HWDGE 2D loads serialize on one SDMA engine (~15us).
"""

import numpy as np

import concourse.bacc as bacc
import concourse.mybir as mybir
import concourse.tile as tile
from concourse.bass_utils import run_bass_kernel_spmd

N_CORES = 8
M_FULL = 4096
MC = M_FULL // N_CORES  # 512 paths per core
D = 100
DP1 = D + 1  # 101
H = 256  # hidden width
F32 = mybir.dt.float32
F16 = mybir.dt.float16

BS = 10  # hessian block size
NJB = D // BS  # 10 j-blocks
# packed block-upper-triangle: j-block jb covers columns k in [10*jb, 100)
WIDTHS = [D - BS * jb for jb in range(NJB)]  # 100, 90, ..., 10
OFFS = np.cumsum([0] + [BS * w for w in WIDTHS]).tolist()  # packed offsets
PCOLS = OFFS[-1]  # 5500
PSCALE = 1024.0  # 2^10: keeps fp16 P entries in normal range

NCHUNK = 500  # matmul free-dim (<=512 fp32 PSUM bank)
NCH = PCOLS // NCHUNK  # 11 chunks
DMA_GRPS = [(0, 4), (4, 8), (8, 11)]  # chunk ranges per output DMA
NM = MC // 128  # 4 m-chunks of 128 paths

# set by test harness to profile; kernel() records exec time here
TRACE = False
LAST_EXEC_NS = None

_CACHE = {}


def _build():
    nc = bacc.Bacc(None, target_bir_lowering=False, debug=False)
    sin_f = mybir.ActivationFunctionType.Sin
    copy_f = mybir.ActivationFunctionType.Copy
    mult = mybir.AluOpType.mult

    # One packed input, (128, 976):
    #   cols 0:768    [XT | W1T] content on partitions 0:101 (rest zero)
    #   cols 768:872  [W1[0:128] | b1 | W2 | b2pad]
    #   cols 872:976  [W1[128:256] | b1 | W2 | 0]
    CBW = DP1 + 3
    in_d = nc.dram_tensor("IN", [128, MC + H + 2 * CBW], F32, kind="ExternalInput")

    u_d = nc.dram_tensor("u", [1, MC], F32, kind="ExternalOutput")
    gp_d = nc.dram_tensor("GP", [128, NM * DP1], F32, kind="ExternalOutput")
    # D2P ships as fp16: its accuracy is fp16-limited by the matmul anyway,
    # and halving the dominant output DMA is worth ~15us; host upcasts.
    d2_d = nc.dram_tensor("D2P", [MC, PCOLS], F16, kind="ExternalOutput")

    with tile.TileContext(nc) as tc:
        with (
            tc.tile_pool(name="const", bufs=1) as const,
            tc.tile_pool(name="work", bufs=2) as work,
            tc.tile_pool(name="stage", bufs=4) as stage_p,
            tc.tile_pool(name="psA", bufs=2, space="PSUM") as psA,
        ):
            # ---- load inputs: one sprayed DMA ----
            inp = const.tile([128, MC + H + 2 * CBW], F32)
            nc.gpsimd.dma_start(inp[:], in_d[:])
            ca = inp[0:DP1, :]
            cb = [inp[:, MC + H + k * CBW : MC + H + (k + 1) * CBW] for k in range(2)]
            xt = ca[:, 0:MC]
            w1t = ca[:, MC : MC + H]
            w1 = [cb[k][:, 0:DP1] for k in range(2)]
            b1c = [cb[k][:, DP1 : DP1 + 1] for k in range(2)]
            w2c = [cb[k][:, DP1 + 1 : DP1 + 2] for k in range(2)]
            b2t = cb[0][0:1, DP1 + 2 : DP1 + 3]

            pihalf = const.tile([128, 1], F32)
            nc.vector.memset(pihalf[:], float(np.pi / 2))

            # w1w2 = W2 o W1 rows (g matmul rhs); fp16 V and -1024*W2*V for P
            w1w2 = [const.tile([128, DP1], F32, tag=f"w1w2_{k}", name=f"w1w2_{k}") for k in range(2)]
            vv16 = [const.tile([128, D], F16, tag=f"vv16_{k}", name=f"vv16_{k}") for k in range(2)]
            nw2v16 = [const.tile([128, D], F16, tag=f"nw2v16_{k}", name=f"nw2v16_{k}") for k in range(2)]
            for k in range(2):
                nc.vector.tensor_scalar_mul(w1w2[k][:], w1[k][:], w2c[k])
                nc.vector.tensor_copy(vv16[k][:], w1[k][:, 1:DP1])
                nc.vector.tensor_scalar_mul(
                    nw2v16[k][:], w1w2[k][:, 1:DP1], -PSCALE
                )

            # trig outputs
            sh16 = [const.tile([128, MC], F16, tag=f"sh16_{k}", name=f"sh16_{k}") for k in range(2)]
            sin32 = [const.tile([128, MC], F32, tag=f"sin32_{k}", name=f"sin32_{k}") for k in range(2)]
            cos32 = [const.tile([128, MC], F32, tag=f"cos32_{k}", name=f"cos32_{k}") for k in range(2)]
            inv2pi = float(1.0 / (2.0 * np.pi))
            twopi = float(2.0 * np.pi)

            # ---- trig: Z^T = W1 @ [t,X]^T; sin/cos via range-reduced Sin ----
            for k in range(2):
                ztp = psA.tile([128, MC], F32, tag="zt")
                nc.tensor.matmul(
                    ztp[:], w1t[:, k * 128 : (k + 1) * 128], xt[:],
                    start=True, stop=True,
                )
                y = work.tile([128, MC], F32, tag="y")
                nc.vector.tensor_scalar_add(y[:], ztp[:], b1c[k])
                ki = work.tile([128, MC], mybir.dt.int32, tag="ki")
                nc.vector.tensor_scalar(
                    out=ki[:], in0=y[:], scalar1=inv2pi, scalar2=None, op0=mult
                )
                kf = work.tile([128, MC], F32, tag="kf")
                nc.vector.tensor_scalar(
                    out=kf[:], in0=ki[:], scalar1=twopi, scalar2=None, op0=mult
                )
                w = work.tile([128, MC], F32, tag="wred")
                nc.vector.tensor_tensor(
                    out=w[:], in0=y[:], in1=kf[:], op=mybir.AluOpType.subtract
                )
                nc.scalar.activation(sh16[k][:], w[:], sin_f)
                nc.scalar.activation(sin32[k][:], w[:], sin_f)
                # cos(z) = sin(w + pi/2), wrapped down a period if w > pi/2
                hi = work.tile([128, MC], F32, tag="hi")
                nc.vector.tensor_scalar(
                    out=hi[:], in0=w[:], scalar1=float(np.pi / 2), scalar2=-twopi,
                    op0=mybir.AluOpType.is_gt, op1=mult,
                )
                wc = work.tile([128, MC], F32, tag="wc")
                nc.vector.tensor_tensor(
                    out=wc[:], in0=w[:], in1=hi[:], op=mybir.AluOpType.add
                )
                nc.scalar.activation(cos32[k][:], wc[:], sin_f, bias=pihalf[:])

            # packed block-triangle P (fp16, persistent); built just-in-time
            # inside the first m-chunk's groups so copies aren't starved.
            PP = [const.tile([128, PCOLS], F16, tag=f"PP_{k}", name=f"PP_{k}") for k in range(2)]

            def build_pp(jbs):
                for k in range(2):
                    for jb in jbs:
                        wjb = WIDTHS[jb]
                        js = slice(jb * BS, (jb + 1) * BS)
                        nc.vector.tensor_tensor(
                            out=PP[k][:, OFFS[jb] : OFFS[jb] + BS * wjb].rearrange(
                                "p (j l) -> p j l", l=wjb
                            ),
                            in0=nw2v16[k][:, js, None].to_broadcast([128, BS, wjb]),
                            in1=vv16[k][:, None, jb * BS : D].to_broadcast([128, BS, wjb]),
                            op=mult,
                        )

            GRP_JBS = {0: [0, 1, 2], 4: [3, 4], 8: [5, 6, 7, 8, 9]}

            # ---- Hessian: D2P[m, c] = (1/1024) sum_h sin16[h,m] * PP[h, c],
            # interleaved with the small u and g matmuls so the big output
            # DMA starts as early as possible and u/g fill PE copy-gaps.
            g_all = const.tile([128, NM * DP1], F32)

            def emit_hess(m):
                ms = slice(m * 128, (m + 1) * 128)
                for g0, g1 in DMA_GRPS:
                    if m == 0:
                        build_pp(GRP_JBS[g0])
                    gcols = (g1 - g0) * NCHUNK
                    st = stage_p.tile([128, 2000], F16, tag="stage", name=f"st_{m}_{g0}")
                    pss = [
                        psA.tile([128, NCHUNK], F32, tag="hess", bufs=5,
                                 name=f"ps_{m}_{c}")
                        for c in range(g0, g1)
                    ]
                    for k in range(2):
                        for i, c in enumerate(range(g0, g1)):
                            cs = slice(c * NCHUNK, (c + 1) * NCHUNK)
                            nc.tensor.matmul(
                                pss[i][:], sh16[k][:, ms], PP[k][:, cs],
                                start=(k == 0), stop=(k == 1),
                            )
                    for i, c in enumerate(range(g0, g1)):
                        ss = slice(i * NCHUNK, (i + 1) * NCHUNK)
                        if i == 0:
                            nc.vector.tensor_scalar_mul(
                                st[:, ss], pss[i][:], 1.0 / PSCALE
                            )
                        else:
                            nc.scalar.activation(
                                st[:, ss], pss[i][:], copy_f, scale=1.0 / PSCALE
                            )
                    nc.sync.dma_start(
                        d2_d[ms, g0 * NCHUNK : g1 * NCHUNK], st[:, 0:gcols]
                    )

            def emit_u():
                up = psA.tile([1, MC], F32, tag="ug", bufs=1, name="up")
                for k in range(2):
                    nc.tensor.matmul(
                        up[:], w2c[k], sin32[k][:], start=(k == 0), stop=(k == 1)
                    )
                u_sb = work.tile([1, MC], F32, tag="usb", name="u_sb")
                nc.vector.tensor_scalar_add(u_sb[:], up[:], b2t)
                nc.sync.dma_start(u_d[:], u_sb[:])

            def emit_g(m):
                ms = slice(m * 128, (m + 1) * 128)
                gp = psA.tile([128, DP1], F32, tag="ug", bufs=1, name=f"gp_{m}")
                for k in range(2):
                    nc.tensor.matmul(
                        gp[:], cos32[k][:, ms], w1w2[k][:], start=(k == 0), stop=(k == 1)
                    )
                nc.vector.tensor_copy(g_all[:, m * DP1 : (m + 1) * DP1], gp[:])

            emit_hess(0)
            emit_u()
            emit_hess(1)
            emit_g(0)
            emit_g(1)
            emit_hess(2)
            emit_g(2)
            emit_g(3)
            nc.sync.dma_start(gp_d[:], g_all[:])
            emit_hess(3)

    nc.compile()
    return nc


def kernel(t, X, W1, b1, W2, b2):
    global LAST_EXEC_NS
    t = np.ascontiguousarray(np.asarray(t, dtype=np.float32))
    X = np.ascontiguousarray(np.asarray(X, dtype=np.float32))
    W1 = np.ascontiguousarray(np.asarray(W1, dtype=np.float32))
    b1 = np.asarray(b1, dtype=np.float32).reshape(H)
    W2 = np.asarray(W2, dtype=np.float32).reshape(H)
    b2 = np.asarray(b2, dtype=np.float32).reshape(1)

    xaug_t = np.concatenate([t, X], axis=1).T  # (101, 4096)
    w1t = W1.T  # (101, 256)

    CBW = DP1 + 3
    base = np.zeros((128, MC + H + 2 * CBW), dtype=np.float32)
    base[0:DP1, MC : MC + H] = w1t
    for k in range(2):
        c0 = MC + H + k * CBW
        base[:, c0 : c0 + DP1] = W1[k * 128 : (k + 1) * 128, :]
        base[:, c0 + DP1] = b1[k * 128 : (k + 1) * 128]
        base[:, c0 + DP1 + 1] = W2[k * 128 : (k + 1) * 128]
    base[0, MC + H + DP1 + 2] = b2[0]

    if "nc" not in _CACHE:
        _CACHE["nc"] = _build()
    nc = _CACHE["nc"]

    in_maps = []
    for i in range(N_CORES):
        pk = base.copy()
        pk[0:DP1, 0:MC] = xaug_t[:, i * MC : (i + 1) * MC]
        in_maps.append({"IN": pk})

    res = run_bass_kernel_spmd(nc, in_maps, list(range(N_CORES)), trace=TRACE)
    LAST_EXEC_NS = res.exec_time_ns

    u = np.concatenate(
        [res.results[i]["u"].reshape(MC, 1) for i in range(N_CORES)], axis=0
    )
    g = np.concatenate(
        [
            res.results[i]["GP"].reshape(128, NM, DP1).transpose(1, 0, 2).reshape(MC, DP1)
            for i in range(N_CORES)
        ],
        axis=0,
    )
    dudt = np.ascontiguousarray(g[:, 0:1])
    dudx = np.ascontiguousarray(g[:, 1:DP1])
    packed = np.concatenate(
        [res.results[i]["D2P"].astype(np.float32) for i in range(N_CORES)], axis=0
    )

    # unpack block-upper-triangle and mirror (Hessian is symmetric)
    d2 = np.empty((M_FULL, D, D), dtype=np.float32)
    for jb in range(NJB):
        wjb = WIDTHS[jb]
        blk = packed[:, OFFS[jb] : OFFS[jb] + BS * wjb].reshape(M_FULL, BS, wjb)
        d2[:, jb * BS : (jb + 1) * BS, jb * BS : D] = blk
    for jb in range(1, NJB):
        for kb in range(jb):
            d2[:, jb * BS : (jb + 1) * BS, kb * BS : (kb + 1) * BS] = d2[
                :, kb * BS : (kb + 1) * BS, jb * BS : (jb + 1) * BS
            ].transpose(0, 2, 1)
    return u, dudx, dudt, d2
